# revision 12
# baseline (speedup 1.0000x reference)
"""MoE (8 experts, top-2, sigmoid router, SwiGLU + shared expert) on 8 TRN2 cores.

Strategy: expert-parallel with host-side token dispatch/combine (the
all-to-all of the sharding hint realized through the full-IO contract).
The host computes the router (fp64 sigmoid scores + top-2 selection),
gathers each expert's assigned tokens, pre-scales them by their routing
score (matmul linearity: silu(W1 @ (s*x)) == silu(s*(W1 @ x)), which the
reference itself relies on), and pads to a 16-granular capacity C. Core
e runs expert e's SwiGLU over its C gathered tokens plus the shared
expert over a 256-token shard; the host scatter-adds routed outputs into
the shared-expert output.

Kernel structure (all matmuls full-128 contraction, bf16):
 - up-projections computed transposed (hidden on PSUM partitions,
   tokens on the free axis) so no on-chip transposes are needed and
   token-capacity waste costs only C, not round-up-to-128 tiles;
 - down-projection keeps w2 stationary and moves h, producing y
   transposed ([dim_chunk, tokens]), again free-axis == tokens;
 - latency-critical weight stream + gathered x ride the SP DMA queue;
   bulk prefetch (w2, shared weights, shard x) and y writebacks ride
   the Activation DMA queue so they never head-of-line block the
   stream that feeds the TensorEngine.
"""
import numpy as np
import ml_dtypes

import concourse.bass as bass
import concourse.tile as tile
from concourse import bacc, mybir
from concourse.bass_utils import run_bass_kernel_spmd

P = 128
N_CORES = 8
SLEN = 2048
DIM = 2048
HID = 1024
E = 8
TOP_K = 2
TOKS = SLEN // N_CORES         # 256 shared-expert tokens per core
DC = DIM // P                  # 16 contraction chunks over dim
HC = HID // P                  # 8 chunks over hidden
BF16 = mybir.dt.bfloat16
F32 = mybir.dt.float32

_CACHE: dict = {}


def _chunks(T):
    """Token chunks along the matmul free axis; each must fit a PSUM bank
    (<=512 fp32). Always two chunks so the A/B tile pairs ping-pong and
    the next iteration's matmuls never wait on this one's act/copy."""
    cA = -(-T // 2 // 8) * 8
    return [(0, cA, "A"), (cA, T - cA, "B")]


def _build(C):
    nc = bacc.Bacc("TRN2", target_bir_lowering=False, debug=False,
                   num_devices=N_CORES)

    xg_d = nc.dram_tensor("xg", [P, DC, C], BF16, kind="ExternalInput").ap()
    xs_d = nc.dram_tensor("xs", [P, DC, TOKS], BF16, kind="ExternalInput").ap()
    # up-proj weights: [HC, P, DC*P]; [hc, p, dc*128+f] = wT[dc*128+p, hc*128+f]
    w1_d = nc.dram_tensor("w1", [HC, P, DC * P], BF16, kind="ExternalInput").ap()
    w3_d = nc.dram_tensor("w3", [HC, P, DC * P], BF16, kind="ExternalInput").ap()
    sw1_d = nc.dram_tensor("sw1", [HC, P, DC * P], BF16, kind="ExternalInput").ap()
    sw3_d = nc.dram_tensor("sw3", [HC, P, DC * P], BF16, kind="ExternalInput").ap()
    # down-proj weights: [P, HC, DIM]; [p, hc, d] = w2T[hc*128+p, d]
    w2_d = nc.dram_tensor("w2", [P, HC, DIM], BF16, kind="ExternalInput").ap()
    sw2_d = nc.dram_tensor("sw2", [P, HC, DIM], BF16, kind="ExternalInput").ap()
    # outputs transposed: [dc, p, tok] = y[tok, dc*128+p]
    yg_d = nc.dram_tensor("yg", [DC, P, C], F32, kind="ExternalOutput").ap()
    ys_d = nc.dram_tensor("ys", [DC, P, TOKS], F32, kind="ExternalOutput").ap()

    with tile.TileContext(nc) as tc:
        with tc.tile_pool(name="xc", bufs=1) as xpool, \
             tc.tile_pool(name="w2c", bufs=1) as w2pool, \
             tc.tile_pool(name="h", bufs=1) as hpool, \
             tc.tile_pool(name="wup", bufs=3) as wup, \
             tc.tile_pool(name="up", bufs=1, space="PSUM") as upps, \
             tc.tile_pool(name="dn", bufs=1, space="PSUM") as dnps, \
             tc.tile_pool(name="tmp", bufs=2) as tmp, \
             tc.tile_pool(name="yst", bufs=4) as yst:

            xg_sb = xpool.tile([P, DC, C], BF16, tag="xg")
            xs_sb = xpool.tile([P, DC, TOKS], BF16, tag="xs")
            w2_sb = w2pool.tile([P, HC, DIM], BF16, tag="w2")
            sw2_sb = w2pool.tile([P, HC, DIM], BF16, tag="sw2")

            def load_up_w(w1d, w3d, hc):
                w1s = wup.tile([P, DC * P], BF16, tag="w1s")
                w3s = wup.tile([P, DC * P], BF16, tag="w3s")
                half = DC * P // 2
                nc.sync.dma_start(w1s[:, :half], w1d[hc, :, :half])
                nc.sync.dma_start(w1s[:, half:], w1d[hc, :, half:])
                nc.sync.dma_start(w3s[:, :half], w3d[hc, :, :half])
                nc.sync.dma_start(w3s[:, half:], w3d[hc, :, half:])
                return w1s, w3s

            # ---- head: shard x and shared hc0 weights only (1.5 MB), so the
            # PE starts on the shared expert while the 2.2 MB gathered x and
            # routed weights stream in behind
            for g in range(4):
                nc.sync.dma_start(xs_sb[:, 4 * g:4 * (g + 1), :],
                                  xs_d[:, 4 * g:4 * (g + 1), :])
            sw_cur = load_up_w(sw1_d, sw3_d, 0)

            def up_phase(T, x_sb, w1d, w3d, hT, w_first, bulk):
                w = w_first
                for hc in range(HC):
                    w_nxt = load_up_w(w1d, w3d, hc + 1) if hc + 1 < HC else None
                    w1s, w3s = w
                    for (t0, tn, cid) in _chunks(T):
                        pg = upps.tile([P, 512], F32, tag=f"pg{cid}",
                                       name=f"pg{cid}")
                        pu = upps.tile([P, 512], F32, tag=f"pu{cid}",
                                       name=f"pu{cid}")
                        for dc in range(DC):
                            nc.tensor.matmul(
                                pg[:, :tn], w1s[:, dc * P:(dc + 1) * P],
                                x_sb[:, dc, t0:t0 + tn],
                                start=(dc == 0), stop=(dc == DC - 1))
                        for dc in range(DC):
                            nc.tensor.matmul(
                                pu[:, :tn], w3s[:, dc * P:(dc + 1) * P],
                                x_sb[:, dc, t0:t0 + tn],
                                start=(dc == 0), stop=(dc == DC - 1))
                        tsg = tmp.tile([P, 512], BF16, tag=f"tsg{cid}")
                        nc.scalar.activation(tsg[:, :tn], pg[:, :tn],
                                             mybir.ActivationFunctionType.Silu)
                        nc.vector.tensor_mul(hT[:, hc, t0:t0 + tn],
                                             tsg[:, :tn], pu[:, :tn])
                    # bulk prefetch rides the ACT queue behind the silu ops
                    for dma in bulk.pop(hc, []):
                        dma()
                    w = w_nxt

            def down_phase(T, hT, w2sb, y_d, ytag, bulk=None):
                for dcD in range(DC):
                    ysb = yst.tile([P, T], F32, tag=ytag)
                    for (t0, tn, cid) in _chunks(T):
                        py = dnps.tile([P, 512], F32, tag=f"py{cid}{dcD % 2}",
                                       name=f"py{cid}{dcD % 2}")
                        for hc in range(HC):
                            nc.tensor.matmul(
                                py[:, :tn],
                                w2sb[:, hc, dcD * P:(dcD + 1) * P],
                                hT[:, hc, t0:t0 + tn],
                                start=(hc == 0), stop=(hc == HC - 1))
                        nc.vector.tensor_copy(ysb[:, t0:t0 + tn], py[:, :tn])
                    nc.scalar.dma_start(y_d[dcD], ysb[:])
                    if bulk:
                        for dma in bulk.pop(dcD, []):
                            dma()

            h_r = hpool.tile([P, HC, C], BF16, tag="hr")
            h_s = hpool.tile([P, HC, TOKS], BF16, tag="hs")

            # During shared-up: gathered x + routed hc0 weights on the SP
            # queue. During routed-up: w2/sw2 bulk on the ACT queue.
            rw_box = []
            bulk_sup = {hc: [lambda g=hc: nc.sync.dma_start(
                            xg_sb[:, 4 * g:4 * (g + 1), :],
                            xg_d[:, 4 * g:4 * (g + 1), :])]
                        for hc in range(4)}
            bulk_sup[4] = [lambda: rw_box.append(load_up_w(w1_d, w3_d, 0))]
            bulk_rup = {hc: [lambda h2=h2: nc.scalar.dma_start(
                            w2_sb[:, h2, :], w2_d[:, h2, :])
                            for h2 in (2 * (hc - 1), 2 * (hc - 1) + 1)]
                        for hc in range(1, 5)}
            bulk_rup[5] = [lambda: nc.scalar.dma_start(sw2_sb[:, 0:4, :],
                                                       sw2_d[:, 0:4, :])]
            bulk_rup[6] = [lambda: nc.scalar.dma_start(sw2_sb[:, 4:8, :],
                                                       sw2_d[:, 4:8, :])]
            up_phase(TOKS, xs_sb, sw1_d, sw3_d, h_s, sw_cur, bulk_sup)
            up_phase(C, xg_sb, w1_d, w3_d, h_r, rw_box[0], bulk_rup)
            down_phase(TOKS, h_s, sw2_sb, ys_d, "yss")
            down_phase(C, h_r, w2_sb, yg_d, "ysr")

    nc.compile()
    return nc


def _get_nc(C):
    key = ("nc", C)
    if key not in _CACHE:
        _CACHE[key] = _build(C)
    return _CACHE[key]


def _bf16(a):
    return np.ascontiguousarray(a.astype(ml_dtypes.bfloat16))


def _up_layout(wT):
    # wT: [DIM, HID] (contraction-major) -> [HC, P, DC*P]
    return _bf16(wT.reshape(DC, P, HC, P).transpose(2, 1, 0, 3)
                 .reshape(HC, P, DC * P))


def _dn_layout(wT):
    # wT: [HID, DIM] -> [P, HC, DIM]
    return _bf16(wT.reshape(HC, P, DIM).transpose(1, 0, 2))


def _x_layout(xrows, T):
    # xrows: [n, DIM] bf16 -> [P, DC, T] with zero padding
    n = xrows.shape[0]
    out = np.zeros((P, DC, T), dtype=ml_dtypes.bfloat16)
    out[:, :, :n] = xrows.T.reshape(DC, P, n).transpose(1, 0, 2)
    return out


def kernel(x, gate, expert_bias, w1, w2, w3, sw1, sw2, sw3, _want_results=False):
    x = np.asarray(x, dtype=np.float32)
    gate = np.asarray(gate, dtype=np.float32)
    expert_bias = np.asarray(expert_bias, dtype=np.float32)

    xt = x.reshape(SLEN, DIM)
    # ---- host router: fp64 scores, top-2 on scores + bias, raw-score weights
    logits = xt.astype(np.float64) @ gate.astype(np.float64)
    scores = 1.0 / (1.0 + np.exp(-logits))
    sel = np.argsort(-(scores + expert_bias.astype(np.float64)[None, :]),
                     axis=1, kind="stable")[:, :TOP_K]

    xb = xt.astype(ml_dtypes.bfloat16)
    tok_lists, s_lists = [], []
    maxcnt = 0
    for e in range(E):
        toks = np.nonzero((sel == e).any(axis=1))[0]
        tok_lists.append(toks)
        s_lists.append(scores[toks, e].astype(np.float32))
        maxcnt = max(maxcnt, len(toks))
    C = max(TOKS, -(-maxcnt // 8) * 8)

    w1t = np.asarray(w1, np.float32).transpose(0, 2, 1)   # (E, DIM, HID)
    w3t = np.asarray(w3, np.float32).transpose(0, 2, 1)
    w2t = np.asarray(w2, np.float32).transpose(0, 2, 1)   # (E, HID, DIM)
    sw1_l = _up_layout(np.asarray(sw1, np.float32).T)
    sw3_l = _up_layout(np.asarray(sw3, np.float32).T)
    sw2_l = _dn_layout(np.asarray(sw2, np.float32).T)

    in_maps = []
    for e in range(E):
        xg_rows = (xb[tok_lists[e]].astype(np.float32)
                   * s_lists[e][:, None]).astype(ml_dtypes.bfloat16)
        in_maps.append({
            "xg": _x_layout(xg_rows, C),
            "xs": _x_layout(xb[e * TOKS:(e + 1) * TOKS], TOKS),
            "w1": _up_layout(w1t[e]), "w3": _up_layout(w3t[e]),
            "w2": _dn_layout(w2t[e]),
            "sw1": sw1_l, "sw3": sw3_l, "sw2": sw2_l,
        })

    nc = _get_nc(C)
    res = run_bass_kernel_spmd(nc, in_maps, list(range(N_CORES)))

    out = np.empty((SLEN, DIM), dtype=np.float32)
    for c in range(N_CORES):
        out[c * TOKS:(c + 1) * TOKS] = (
            res.results[c]["ys"].transpose(2, 0, 1).reshape(TOKS, DIM))
    for e in range(E):
        n = len(tok_lists[e])
        yg = res.results[e]["yg"].transpose(2, 0, 1).reshape(C, DIM)
        out[tok_lists[e]] += yg[:n]
    out = out.reshape(1, 1, SLEN, DIM)
    if _want_results:
        return out, res
    return out


# revision 16
# speedup vs baseline: 1.0131x; 1.0131x over previous
"""MoE (8 experts, top-2, sigmoid router, SwiGLU + shared expert) on 8 TRN2 cores.

Strategy: expert-parallel with host-side token dispatch/combine (the
all-to-all of the sharding hint realized through the full-IO contract).
The host computes the router (fp64 sigmoid scores + top-2 selection),
gathers each expert's assigned tokens, pre-scales them by their routing
score (matmul linearity: silu(W1 @ (s*x)) == silu(s*(W1 @ x)), which the
reference itself relies on), and pads to a 16-granular capacity C. Core
e runs expert e's SwiGLU over its C gathered tokens plus the shared
expert over a 256-token shard; the host scatter-adds routed outputs into
the shared-expert output.

Kernel structure (all matmuls full-128 contraction, bf16):
 - up-projections computed transposed (hidden on PSUM partitions,
   tokens on the free axis) so no on-chip transposes are needed and
   token-capacity waste costs only C, not round-up-to-128 tiles;
 - down-projection keeps w2 stationary and moves h, producing y
   transposed ([dim_chunk, tokens]), again free-axis == tokens;
 - latency-critical weight stream + gathered x ride the SP DMA queue;
   bulk prefetch (w2, shared weights, shard x) and y writebacks ride
   the Activation DMA queue so they never head-of-line block the
   stream that feeds the TensorEngine.
"""
import numpy as np
import ml_dtypes

import concourse.bass as bass
import concourse.tile as tile
from concourse import bacc, mybir
from concourse.bass_utils import run_bass_kernel_spmd

P = 128
N_CORES = 8
SLEN = 2048
DIM = 2048
HID = 1024
E = 8
TOP_K = 2
TOKS = SLEN // N_CORES         # 256 shared-expert tokens per core
DC = DIM // P                  # 16 contraction chunks over dim
HC = HID // P                  # 8 chunks over hidden
BF16 = mybir.dt.bfloat16
F32 = mybir.dt.float32

_CACHE: dict = {}


def _chunks(T):
    """Token chunks along the matmul free axis; each must fit a PSUM bank
    (<=512 fp32). Always two chunks so the A/B tile pairs ping-pong and
    the next iteration's matmuls never wait on this one's act/copy."""
    cA = -(-T // 2 // 8) * 8
    return [(0, cA, "A"), (cA, T - cA, "B")]


def _build(C):
    nc = bacc.Bacc("TRN2", target_bir_lowering=False, debug=False,
                   num_devices=N_CORES)

    xg_d = nc.dram_tensor("xg", [P, DC, C], BF16, kind="ExternalInput").ap()
    xs_d = nc.dram_tensor("xs", [P, DC, TOKS], BF16, kind="ExternalInput").ap()
    # up-proj weights: [HC, P, DC*P]; [hc, p, dc*128+f] = wT[dc*128+p, hc*128+f]
    w1_d = nc.dram_tensor("w1", [HC, P, DC * P], BF16, kind="ExternalInput").ap()
    w3_d = nc.dram_tensor("w3", [HC, P, DC * P], BF16, kind="ExternalInput").ap()
    sw1_d = nc.dram_tensor("sw1", [HC, P, DC * P], BF16, kind="ExternalInput").ap()
    sw3_d = nc.dram_tensor("sw3", [HC, P, DC * P], BF16, kind="ExternalInput").ap()
    # down-proj weights: [P, HC, DIM]; [p, hc, d] = w2T[hc*128+p, d]
    w2_d = nc.dram_tensor("w2", [P, HC, DIM], BF16, kind="ExternalInput").ap()
    sw2_d = nc.dram_tensor("sw2", [P, HC, DIM], BF16, kind="ExternalInput").ap()
    # outputs transposed: [dc, p, tok] = y[tok, dc*128+p]
    yg_d = nc.dram_tensor("yg", [DC, P, C], F32, kind="ExternalOutput").ap()
    ys_d = nc.dram_tensor("ys", [DC, P, TOKS], F32, kind="ExternalOutput").ap()

    with tile.TileContext(nc) as tc:
        with tc.tile_pool(name="xc", bufs=1) as xpool, \
             tc.tile_pool(name="w2c", bufs=1) as w2pool, \
             tc.tile_pool(name="h", bufs=1) as hpool, \
             tc.tile_pool(name="wup", bufs=2) as wup, \
             tc.tile_pool(name="up", bufs=1, space="PSUM") as upps, \
             tc.tile_pool(name="dn", bufs=1, space="PSUM") as dnps, \
             tc.tile_pool(name="tmp", bufs=2) as tmp, \
             tc.tile_pool(name="yst", bufs=4) as yst:

            xg_sb = xpool.tile([P, DC, C], BF16, tag="xg")
            xs_sb = xpool.tile([P, DC, TOKS], BF16, tag="xs")
            w2_sb = w2pool.tile([P, HC, DIM], BF16, tag="w2")
            sw2_sb = w2pool.tile([P, HC, DIM], BF16, tag="sw2")

            def load_up_w(w1d, w3d, hc, tp):
                w1s = wup.tile([P, DC * P], BF16, tag=f"w1s{tp}")
                w3s = wup.tile([P, DC * P], BF16, tag=f"w3s{tp}")
                half = DC * P // 2
                nc.sync.dma_start(w1s[:, :half], w1d[hc, :, :half])
                nc.sync.dma_start(w1s[:, half:], w1d[hc, :, half:])
                nc.sync.dma_start(w3s[:, :half], w3d[hc, :, :half])
                nc.sync.dma_start(w3s[:, half:], w3d[hc, :, half:])
                return w1s, w3s

            # ---- head: shard x and shared hc0 weights first (1.5 MB), so
            # the PE starts on the shared expert while the 2.2 MB gathered x
            # and routed weights stream in behind
            for g in range(4):
                nc.sync.dma_start(xs_sb[:, 4 * g:4 * (g + 1), :],
                                  xs_d[:, 4 * g:4 * (g + 1), :])
            sw_cur = load_up_w(sw1_d, sw3_d, 0, "s")
            nc.sync.dma_start(xg_sb[:, 0:4, :], xg_d[:, 0:4, :])
            nc.sync.dma_start(xg_sb[:, 4:8, :], xg_d[:, 4:8, :])

            def up_iter(st, hc, bulk_dmas):
                if hc + 1 < HC:
                    w_nxt = load_up_w(st["w1d"], st["w3d"], hc + 1, st["tp"])
                w1s, w3s = st["cur"]
                T, x_sb, hT = st["T"], st["x"], st["h"]
                for (t0, tn, cid) in _chunks(T):
                    pg = upps.tile([P, 512], F32, tag=f"pg{cid}",
                                   name=f"pg{cid}")
                    pu = upps.tile([P, 512], F32, tag=f"pu{cid}",
                                   name=f"pu{cid}")
                    for dc in range(DC):
                        nc.tensor.matmul(
                            pg[:, :tn], w1s[:, dc * P:(dc + 1) * P],
                            x_sb[:, dc, t0:t0 + tn],
                            start=(dc == 0), stop=(dc == DC - 1))
                    for dc in range(DC):
                        nc.tensor.matmul(
                            pu[:, :tn], w3s[:, dc * P:(dc + 1) * P],
                            x_sb[:, dc, t0:t0 + tn],
                            start=(dc == 0), stop=(dc == DC - 1))
                    tsg = tmp.tile([P, 512], BF16, tag=f"tsg{cid}")
                    nc.scalar.activation(tsg[:, :tn], pg[:, :tn],
                                         mybir.ActivationFunctionType.Silu)
                    nc.vector.tensor_mul(hT[:, hc, t0:t0 + tn],
                                         tsg[:, :tn], pu[:, :tn])
                for dma in bulk_dmas:
                    dma()
                if hc + 1 < HC:
                    st["cur"] = w_nxt

            def down_phase(T, hT, w2sb, y_d, ytag, bulk=None):
                for dcD in range(DC):
                    ysb = yst.tile([P, T], F32, tag=ytag)
                    for (t0, tn, cid) in _chunks(T):
                        py = dnps.tile([P, 512], F32, tag=f"py{cid}{dcD % 2}",
                                       name=f"py{cid}{dcD % 2}")
                        for hc in range(HC):
                            nc.tensor.matmul(
                                py[:, :tn],
                                w2sb[:, hc, dcD * P:(dcD + 1) * P],
                                hT[:, hc, t0:t0 + tn],
                                start=(hc == 0), stop=(hc == HC - 1))
                        nc.vector.tensor_copy(ysb[:, t0:t0 + tn], py[:, :tn])
                    nc.scalar.dma_start(y_d[dcD], ysb[:])
                    if bulk:
                        for dma in bulk.pop(dcD, []):
                            dma()

            h_r = hpool.tile([P, HC, C], BF16, tag="hr")
            h_s = hpool.tile([P, HC, TOKS], BF16, tag="hs")

            # Interleave shared-up and routed-up iterations: combined PE
            # window per slot (~12 us) comfortably covers both weight
            # streams plus background prefetch, and the head needs only
            # 1.5 MB before the PE starts. Shared runs two slots ahead so
            # routed hc0 has time for the rest of xg + its weights to land.
            S = {"T": TOKS, "x": xs_sb, "h": h_s, "w1d": sw1_d, "w3d": sw3_d,
                 "tp": "s", "cur": sw_cur}
            R = {"T": C, "x": xg_sb, "h": h_r, "w1d": w1_d, "w3d": w3_d,
                 "tp": "r", "cur": None}
            sched = [(S, 0), (S, 1), (R, 0), (S, 2), (R, 1), (S, 3), (R, 2),
                     (S, 4), (R, 3), (S, 5), (R, 4), (S, 6), (R, 5), (S, 7),
                     (R, 6), (R, 7)]
            bulk = {
                0: [lambda: nc.sync.dma_start(xg_sb[:, 8:12, :],
                                              xg_d[:, 8:12, :]),
                    lambda: nc.sync.dma_start(xg_sb[:, 12:16, :],
                                              xg_d[:, 12:16, :]),
                    lambda: R.__setitem__(
                        "cur", load_up_w(w1_d, w3_d, 0, "r"))],
            }
            for slot in range(4, 12):
                h2 = slot - 4
                bulk[slot] = [lambda h2=h2: nc.scalar.dma_start(
                    w2_sb[:, h2, :], w2_d[:, h2, :])]
            bulk[12] = [lambda: nc.scalar.dma_start(sw2_sb[:, 0:4, :],
                                                    sw2_d[:, 0:4, :])]
            bulk[13] = [lambda: nc.scalar.dma_start(sw2_sb[:, 4:8, :],
                                                    sw2_d[:, 4:8, :])]
            for i, (st, hc) in enumerate(sched):
                up_iter(st, hc, bulk.get(i, []))
            down_phase(C, h_r, w2_sb, yg_d, "ysr")
            down_phase(TOKS, h_s, sw2_sb, ys_d, "yss")

    nc.compile()
    return nc


def _get_nc(C):
    key = ("nc", C)
    if key not in _CACHE:
        _CACHE[key] = _build(C)
    return _CACHE[key]


def _bf16(a):
    return np.ascontiguousarray(a.astype(ml_dtypes.bfloat16))


def _up_layout(wT):
    # wT: [DIM, HID] (contraction-major) -> [HC, P, DC*P]
    return _bf16(wT.reshape(DC, P, HC, P).transpose(2, 1, 0, 3)
                 .reshape(HC, P, DC * P))


def _dn_layout(wT):
    # wT: [HID, DIM] -> [P, HC, DIM]
    return _bf16(wT.reshape(HC, P, DIM).transpose(1, 0, 2))


def _x_layout(xrows, T):
    # xrows: [n, DIM] bf16 -> [P, DC, T] with zero padding
    n = xrows.shape[0]
    out = np.zeros((P, DC, T), dtype=ml_dtypes.bfloat16)
    out[:, :, :n] = xrows.T.reshape(DC, P, n).transpose(1, 0, 2)
    return out


def kernel(x, gate, expert_bias, w1, w2, w3, sw1, sw2, sw3, _want_results=False):
    x = np.asarray(x, dtype=np.float32)
    gate = np.asarray(gate, dtype=np.float32)
    expert_bias = np.asarray(expert_bias, dtype=np.float32)

    xt = x.reshape(SLEN, DIM)
    # ---- host router: fp64 scores, top-2 on scores + bias, raw-score weights
    logits = xt.astype(np.float64) @ gate.astype(np.float64)
    scores = 1.0 / (1.0 + np.exp(-logits))
    sel = np.argsort(-(scores + expert_bias.astype(np.float64)[None, :]),
                     axis=1, kind="stable")[:, :TOP_K]

    xb = xt.astype(ml_dtypes.bfloat16)
    tok_lists, s_lists = [], []
    maxcnt = 0
    for e in range(E):
        toks = np.nonzero((sel == e).any(axis=1))[0]
        tok_lists.append(toks)
        s_lists.append(scores[toks, e].astype(np.float32))
        maxcnt = max(maxcnt, len(toks))
    C = max(TOKS, -(-maxcnt // 8) * 8)

    w1t = np.asarray(w1, np.float32).transpose(0, 2, 1)   # (E, DIM, HID)
    w3t = np.asarray(w3, np.float32).transpose(0, 2, 1)
    w2t = np.asarray(w2, np.float32).transpose(0, 2, 1)   # (E, HID, DIM)
    sw1_l = _up_layout(np.asarray(sw1, np.float32).T)
    sw3_l = _up_layout(np.asarray(sw3, np.float32).T)
    sw2_l = _dn_layout(np.asarray(sw2, np.float32).T)

    in_maps = []
    for e in range(E):
        xg_rows = (xb[tok_lists[e]].astype(np.float32)
                   * s_lists[e][:, None]).astype(ml_dtypes.bfloat16)
        in_maps.append({
            "xg": _x_layout(xg_rows, C),
            "xs": _x_layout(xb[e * TOKS:(e + 1) * TOKS], TOKS),
            "w1": _up_layout(w1t[e]), "w3": _up_layout(w3t[e]),
            "w2": _dn_layout(w2t[e]),
            "sw1": sw1_l, "sw3": sw3_l, "sw2": sw2_l,
        })

    nc = _get_nc(C)
    res = run_bass_kernel_spmd(nc, in_maps, list(range(N_CORES)))

    out = np.empty((SLEN, DIM), dtype=np.float32)
    for c in range(N_CORES):
        out[c * TOKS:(c + 1) * TOKS] = (
            res.results[c]["ys"].transpose(2, 0, 1).reshape(TOKS, DIM))
    for e in range(E):
        n = len(tok_lists[e])
        yg = res.results[e]["yg"].transpose(2, 0, 1).reshape(C, DIM)
        out[tok_lists[e]] += yg[:n]
    out = out.reshape(1, 1, SLEN, DIM)
    if _want_results:
        return out, res
    return out


# revision 17
# speedup vs baseline: 1.1680x; 1.1528x over previous
"""MoE (8 experts, top-2, sigmoid router, SwiGLU + shared expert) on 8 TRN2 cores.

Strategy: expert-parallel with host-side token dispatch/combine (the
all-to-all of the sharding hint realized through the full-IO contract).
The host computes the router (fp64 sigmoid scores + top-2 selection),
gathers each expert's assigned tokens, pre-scales them by their routing
score (matmul linearity: silu(W1 @ (s*x)) == silu(s*(W1 @ x)), which the
reference itself relies on), and pads to a 16-granular capacity C. Core
e runs expert e's SwiGLU over its C gathered tokens plus the shared
expert over a 256-token shard; the host scatter-adds routed outputs into
the shared-expert output.

Kernel structure (all matmuls full-128 contraction, bf16):
 - up-projections computed transposed (hidden on PSUM partitions,
   tokens on the free axis) so no on-chip transposes are needed and
   token-capacity waste costs only C, not round-up-to-128 tiles;
 - down-projection keeps w2 stationary and moves h, producing y
   transposed ([dim_chunk, tokens]), again free-axis == tokens;
 - latency-critical weight stream + gathered x ride the SP DMA queue;
   bulk prefetch (w2, shared weights, shard x) and y writebacks ride
   the Activation DMA queue so they never head-of-line block the
   stream that feeds the TensorEngine.
"""
import numpy as np
import ml_dtypes

import concourse.bass as bass
import concourse.tile as tile
from concourse import bacc, mybir
from concourse.bass_utils import run_bass_kernel_spmd

P = 128
N_CORES = 8
SLEN = 2048
DIM = 2048
HID = 1024
E = 8
TOP_K = 2
TOKS = SLEN // N_CORES         # 256 shared-expert tokens per core
DC = DIM // P                  # 16 contraction chunks over dim
HC = HID // P                  # 8 chunks over hidden
BF16 = mybir.dt.bfloat16
F32 = mybir.dt.float32

_CACHE: dict = {}


def _chunks(T):
    """Token chunks along the matmul free axis; each must fit a PSUM bank
    (<=512 fp32). Always two chunks so the A/B tile pairs ping-pong and
    the next iteration's matmuls never wait on this one's act/copy."""
    cA = -(-T // 2 // 8) * 8
    return [(0, cA, "A"), (cA, T - cA, "B")]


def _build(C):
    nc = bacc.Bacc("TRN2", target_bir_lowering=False, debug=False,
                   num_devices=N_CORES)

    xg_d = nc.dram_tensor("xg", [P, DC, C], BF16, kind="ExternalInput").ap()
    xs_d = nc.dram_tensor("xs", [P, DC, TOKS], BF16, kind="ExternalInput").ap()
    # up-proj weights: [HC, P, DC*P]; [hc, p, dc*128+f] = wT[dc*128+p, hc*128+f]
    w1_d = nc.dram_tensor("w1", [HC, P, DC * P], BF16, kind="ExternalInput").ap()
    w3_d = nc.dram_tensor("w3", [HC, P, DC * P], BF16, kind="ExternalInput").ap()
    sw1_d = nc.dram_tensor("sw1", [HC, P, DC * P], BF16, kind="ExternalInput").ap()
    sw3_d = nc.dram_tensor("sw3", [HC, P, DC * P], BF16, kind="ExternalInput").ap()
    # down-proj weights: [P, HC, DIM]; [p, hc, d] = w2T[hc*128+p, d]
    w2_d = nc.dram_tensor("w2", [P, HC, DIM], BF16, kind="ExternalInput").ap()
    sw2_d = nc.dram_tensor("sw2", [P, HC, DIM], BF16, kind="ExternalInput").ap()
    # outputs transposed: [dc, p, tok] = y[tok, dc*128+p]
    yg_d = nc.dram_tensor("yg", [DC, P, C], F32, kind="ExternalOutput").ap()
    ys_d = nc.dram_tensor("ys", [DC, P, TOKS], F32, kind="ExternalOutput").ap()

    with tile.TileContext(nc) as tc:
        with tc.tile_pool(name="xc", bufs=1) as xpool, \
             tc.tile_pool(name="w2c", bufs=1) as w2pool, \
             tc.tile_pool(name="h", bufs=1) as hpool, \
             tc.tile_pool(name="wup", bufs=2) as wup, \
             tc.tile_pool(name="up", bufs=1, space="PSUM") as upps, \
             tc.tile_pool(name="dn", bufs=1, space="PSUM") as dnps, \
             tc.tile_pool(name="tmp", bufs=2) as tmp, \
             tc.tile_pool(name="yst", bufs=4) as yst:

            xg_sb = xpool.tile([P, DC, C], BF16, tag="xg")
            xs_sb = xpool.tile([P, DC, TOKS], BF16, tag="xs")
            w2_sb = w2pool.tile([P, HC, DIM], BF16, tag="w2")
            sw2_sb = w2pool.tile([P, HC, DIM], BF16, tag="sw2")

            def load_up_w(w1d, w3d, hc, tp):
                w1s = wup.tile([P, DC * P], BF16, tag=f"w1s{tp}")
                w3s = wup.tile([P, DC * P], BF16, tag=f"w3s{tp}")
                half = DC * P // 2
                nc.sync.dma_start(w1s[:, :half], w1d[hc, :, :half])
                nc.sync.dma_start(w1s[:, half:], w1d[hc, :, half:])
                nc.sync.dma_start(w3s[:, :half], w3d[hc, :, :half])
                nc.sync.dma_start(w3s[:, half:], w3d[hc, :, half:])
                return w1s, w3s

            # ---- head: shard x and shared hc0 weights first (1.5 MB), so
            # the PE starts on the shared expert while the 2.2 MB gathered x
            # and routed weights stream in behind
            for g in range(4):
                nc.sync.dma_start(xs_sb[:, 4 * g:4 * (g + 1), :],
                                  xs_d[:, 4 * g:4 * (g + 1), :])
            sw_cur = load_up_w(sw1_d, sw3_d, 0, "s")
            nc.sync.dma_start(xg_sb[:, 0:4, :], xg_d[:, 0:4, :])
            nc.sync.dma_start(xg_sb[:, 4:8, :], xg_d[:, 4:8, :])

            def up_iter(st, hc, bulk_dmas):
                if hc + 1 < HC:
                    w_nxt = load_up_w(st["w1d"], st["w3d"], hc + 1, st["tp"])
                w1s, w3s = st["cur"]
                T, x_sb, hT = st["T"], st["x"], st["h"]
                for (t0, tn, cid) in _chunks(T):
                    pg = upps.tile([P, 512], F32, tag=f"pg{cid}",
                                   name=f"pg{cid}")
                    pu = upps.tile([P, 512], F32, tag=f"pu{cid}",
                                   name=f"pu{cid}")
                    for dc in range(DC):
                        nc.tensor.matmul(
                            pg[:, :tn], w1s[:, dc * P:(dc + 1) * P],
                            x_sb[:, dc, t0:t0 + tn],
                            start=(dc == 0), stop=(dc == DC - 1))
                    for dc in range(DC):
                        nc.tensor.matmul(
                            pu[:, :tn], w3s[:, dc * P:(dc + 1) * P],
                            x_sb[:, dc, t0:t0 + tn],
                            start=(dc == 0), stop=(dc == DC - 1))
                    tsg = tmp.tile([P, 512], BF16, tag=f"tsg{cid}")
                    nc.scalar.activation(tsg[:, :tn], pg[:, :tn],
                                         mybir.ActivationFunctionType.Silu)
                    nc.vector.tensor_mul(hT[:, hc, t0:t0 + tn],
                                         tsg[:, :tn], pu[:, :tn])
                for dma in bulk_dmas:
                    dma()
                if hc + 1 < HC:
                    st["cur"] = w_nxt

            def down_phase(T, hT, w2sb, y_d, ytag, bulk=None):
                for dcD in range(DC):
                    ysb = yst.tile([P, T], F32, tag=ytag)
                    for (t0, tn, cid) in _chunks(T):
                        py = dnps.tile([P, 512], F32, tag=f"py{cid}{dcD % 2}",
                                       name=f"py{cid}{dcD % 2}")
                        for hc in range(HC):
                            nc.tensor.matmul(
                                py[:, :tn],
                                w2sb[:, hc, dcD * P:(dcD + 1) * P],
                                hT[:, hc, t0:t0 + tn],
                                start=(hc == 0), stop=(hc == HC - 1))
                        nc.vector.tensor_copy(ysb[:, t0:t0 + tn], py[:, :tn])
                    nc.scalar.dma_start(y_d[dcD], ysb[:])
                    if bulk:
                        for dma in bulk.pop(dcD, []):
                            dma()

            h_r = hpool.tile([P, HC, C], BF16, tag="hr")
            h_s = hpool.tile([P, HC, TOKS], BF16, tag="hs")

            # Interleave shared-up and routed-up iterations: combined PE
            # window per slot (~12 us) comfortably covers both weight
            # streams plus background prefetch, and the head needs only
            # 1.5 MB before the PE starts. Shared runs two slots ahead so
            # routed hc0 has time for the rest of xg + its weights to land.
            S = {"T": TOKS, "x": xs_sb, "h": h_s, "w1d": sw1_d, "w3d": sw3_d,
                 "tp": "s", "cur": sw_cur}
            R = {"T": C, "x": xg_sb, "h": h_r, "w1d": w1_d, "w3d": w3_d,
                 "tp": "r", "cur": None}
            sched = [(S, 0), (S, 1), (R, 0), (S, 2), (R, 1), (S, 3), (R, 2),
                     (S, 4), (R, 3), (S, 5), (R, 4), (S, 6), (R, 5), (S, 7),
                     (R, 6), (R, 7)]
            # All bulk prefetch rides the SP queue: the tile scheduler keeps
            # SP DMAs in emission order, whereas dependency-free DMAs on the
            # ACT queue get hoisted to t=0 and hog the DMA device during the
            # critical head.
            bulk = {
                0: [lambda: nc.sync.dma_start(xg_sb[:, 8:12, :],
                                              xg_d[:, 8:12, :]),
                    lambda: nc.sync.dma_start(xg_sb[:, 12:16, :],
                                              xg_d[:, 12:16, :]),
                    lambda: R.__setitem__(
                        "cur", load_up_w(w1_d, w3_d, 0, "r"))],
            }
            for slot in range(4, 12):
                h2 = slot - 4
                bulk[slot] = [
                    lambda h2=h2: nc.sync.dma_start(w2_sb[:, h2, :],
                                                    w2_d[:, h2, :]),
                    lambda h2=h2: nc.sync.dma_start(sw2_sb[:, h2, :],
                                                    sw2_d[:, h2, :]),
                ]
            for i, (st, hc) in enumerate(sched):
                up_iter(st, hc, bulk.get(i, []))
            down_phase(C, h_r, w2_sb, yg_d, "ysr")
            down_phase(TOKS, h_s, sw2_sb, ys_d, "yss")

    nc.compile()
    return nc


def _get_nc(C):
    key = ("nc", C)
    if key not in _CACHE:
        _CACHE[key] = _build(C)
    return _CACHE[key]


def _bf16(a):
    return np.ascontiguousarray(a.astype(ml_dtypes.bfloat16))


def _up_layout(wT):
    # wT: [DIM, HID] (contraction-major) -> [HC, P, DC*P]
    return _bf16(wT.reshape(DC, P, HC, P).transpose(2, 1, 0, 3)
                 .reshape(HC, P, DC * P))


def _dn_layout(wT):
    # wT: [HID, DIM] -> [P, HC, DIM]
    return _bf16(wT.reshape(HC, P, DIM).transpose(1, 0, 2))


def _x_layout(xrows, T):
    # xrows: [n, DIM] bf16 -> [P, DC, T] with zero padding
    n = xrows.shape[0]
    out = np.zeros((P, DC, T), dtype=ml_dtypes.bfloat16)
    out[:, :, :n] = xrows.T.reshape(DC, P, n).transpose(1, 0, 2)
    return out


def kernel(x, gate, expert_bias, w1, w2, w3, sw1, sw2, sw3, _want_results=False):
    x = np.asarray(x, dtype=np.float32)
    gate = np.asarray(gate, dtype=np.float32)
    expert_bias = np.asarray(expert_bias, dtype=np.float32)

    xt = x.reshape(SLEN, DIM)
    # ---- host router: fp64 scores, top-2 on scores + bias, raw-score weights
    logits = xt.astype(np.float64) @ gate.astype(np.float64)
    scores = 1.0 / (1.0 + np.exp(-logits))
    sel = np.argsort(-(scores + expert_bias.astype(np.float64)[None, :]),
                     axis=1, kind="stable")[:, :TOP_K]

    xb = xt.astype(ml_dtypes.bfloat16)
    tok_lists, s_lists = [], []
    maxcnt = 0
    for e in range(E):
        toks = np.nonzero((sel == e).any(axis=1))[0]
        tok_lists.append(toks)
        s_lists.append(scores[toks, e].astype(np.float32))
        maxcnt = max(maxcnt, len(toks))
    C = max(TOKS, -(-maxcnt // 8) * 8)

    w1t = np.asarray(w1, np.float32).transpose(0, 2, 1)   # (E, DIM, HID)
    w3t = np.asarray(w3, np.float32).transpose(0, 2, 1)
    w2t = np.asarray(w2, np.float32).transpose(0, 2, 1)   # (E, HID, DIM)
    sw1_l = _up_layout(np.asarray(sw1, np.float32).T)
    sw3_l = _up_layout(np.asarray(sw3, np.float32).T)
    sw2_l = _dn_layout(np.asarray(sw2, np.float32).T)

    in_maps = []
    for e in range(E):
        xg_rows = (xb[tok_lists[e]].astype(np.float32)
                   * s_lists[e][:, None]).astype(ml_dtypes.bfloat16)
        in_maps.append({
            "xg": _x_layout(xg_rows, C),
            "xs": _x_layout(xb[e * TOKS:(e + 1) * TOKS], TOKS),
            "w1": _up_layout(w1t[e]), "w3": _up_layout(w3t[e]),
            "w2": _dn_layout(w2t[e]),
            "sw1": sw1_l, "sw3": sw3_l, "sw2": sw2_l,
        })

    nc = _get_nc(C)
    res = run_bass_kernel_spmd(nc, in_maps, list(range(N_CORES)))

    out = np.empty((SLEN, DIM), dtype=np.float32)
    for c in range(N_CORES):
        out[c * TOKS:(c + 1) * TOKS] = (
            res.results[c]["ys"].transpose(2, 0, 1).reshape(TOKS, DIM))
    for e in range(E):
        n = len(tok_lists[e])
        yg = res.results[e]["yg"].transpose(2, 0, 1).reshape(C, DIM)
        out[tok_lists[e]] += yg[:n]
    out = out.reshape(1, 1, SLEN, DIM)
    if _want_results:
        return out, res
    return out
